# revision 1
# baseline (speedup 1.0000x reference)
"""Multi-head attention (B=8, S=1500, E=1024, H=16, D=64) on 8 trn2 NeuronCores.

Sharding: pure data-parallel over batch — core b computes batch element b
end-to-end (no collectives). Host pre-transposes x and the weights so every
device-side matmul has its contraction dim on the SBUF partition axis, and
folds the 1/sqrt(D) scale into Wq/bq and the V-bias into the output bias
(bo_eff = bo + Wo @ bv), so the device kernel never touches bv.

Device pipeline per core (all f32 storage, matmuls run as float32r):
  QT = (Wq/8)^T-proj of x^T   [1024, 1500]  (f-on-partition; bias bq/8 per-partition)
  KT = Wk^T-proj              [1024, 1500]
  V_aug = x @ Wv^T with a ones-column appended per head  [1500, 16*65]
  per (i-chunk, head): scoresT[j, i] via matmul(lhsT=KT_h, rhs=QT_h);
    exp on ACT (no max-subtraction: |scores| <~ 30, safely inside f32);
    causal masking structurally (affine_select zero-fill on diagonal blocks)
    or via an additive mask tensor (general path);
  out^T + rowsums in ONE matmul: lhsT=[V_h | 1] (65 cols), rhs=attnT;
  normalize: recip of the sums row, rank-1 matmul broadcast across 64
    partitions, multiply on eviction -> AO^T;
  yT = Wo^T-proj of AO^T + bo_eff  -> DRAM [1024, 1500], host transposes back.
"""

import sys
import numpy as np
import ml_dtypes

for _p in ("/opt/trn_rl_repo",):
    if _p not in sys.path:
        sys.path.append(_p)

import concourse.bass as bass
import concourse.mybir as mybir
import concourse.tile as tile
from concourse import bacc
from concourse.bass_utils import run_bass_kernel_spmd

F32 = mybir.dt.float32

B, S, E, H, D = 8, 1500, 1024, 16, 64
P = 128
NEG = -1e9


def _chunks(total, step):
    return [(c0, min(step, total - c0)) for c0 in range(0, total, step)]


def _wslices(dram_ap, col0, cols):
    """[E, E] weight -> [P, E//P, cols] AP for a column slice (k on partition)."""
    return dram_ap.rearrange("(kt p) f -> p kt f", p=P)[:, :, col0:col0 + cols]


def build(causal: bool, mm_dt=mybir.dt.bfloat16):
    KT_N = E // P            # k-tiles over the embedding dim
    FT_N = E // P            # f-tiles
    R_CH = _chunks(S, 512)   # i/r chunks
    JB = _chunks(S, P)       # j blocks
    H_LOC = E // D
    nc = bacc.Bacc("TRN2", target_bir_lowering=False, debug=False, num_devices=8)
    MD = mm_dt  # dtype for every matmul operand chain
    NRM = mybir.dt.float32r if MD == mybir.dt.bfloat16 else MD

    xT = nc.dram_tensor("xT", [E, S], MD, kind="ExternalInput")
    wqT = nc.dram_tensor("wqT", [E, E], MD, kind="ExternalInput")
    wkT = nc.dram_tensor("wkT", [E, E], MD, kind="ExternalInput")
    wvT = nc.dram_tensor("wvT", [E, E], MD, kind="ExternalInput")
    woT = nc.dram_tensor("woT", [E, E], MD, kind="ExternalInput")
    bq = nc.dram_tensor("bq", [E], F32, kind="ExternalInput")
    bo = nc.dram_tensor("bo", [E], F32, kind="ExternalInput")
    maskT = None
    if not causal:
        maskT = nc.dram_tensor("maskT", [S, S], F32, kind="ExternalInput")
    yT = nc.dram_tensor("yT", [E, S], F32, kind="ExternalOutput")

    def mm(ap):
        return ap

    nc._allow_low_precision_reason = "low-precision matmul operand chain"
    with tile.TileContext(nc) as tc:
        with (
            tc.tile_pool(name="persist", bufs=1) as pers,
            tc.tile_pool(name="wqkp", bufs=2) as wqkp,
            tc.tile_pool(name="wvp", bufs=1) as wvp,
            tc.tile_pool(name="wop", bufs=1) as wop,
            tc.tile_pool(name="attn", bufs=3) as apool,
            tc.tile_pool(name="small", bufs=3) as spool,
            tc.tile_pool(name="evp", bufs=3) as evp,
            tc.tile_pool(name="psP", bufs=1, space="PSUM") as psP,
            tc.tile_pool(name="psS", bufs=1, space="PSUM") as psS,
            tc.tile_pool(name="psO", bufs=1, space="PSUM") as psO,
            tc.tile_pool(name="psY", bufs=1, space="PSUM") as psY,
        ):
            ones64 = pers.tile([1, D], NRM, name="ones64")
            nc.vector.memset(ones64[:].bitcast(F32), 1.0)
            bq_sb = pers.tile([P, FT_N], F32, name="bq_sb")
            nc.sync.dma_start(out=bq_sb[:], in_=bq.ap().rearrange("(t p) -> p t", p=P))
            bo_sb = pers.tile([P, FT_N], F32, name="bo_sb")
            nc.sync.dma_start(out=bo_sb[:], in_=bo.ap().rearrange("(t p) -> p t", p=P))

            # upper-triangular (incl diag) 0/1 mask for diagonal attn blocks
            tri32 = pers.tile([P, P], F32, name="tri32")
            nc.gpsimd.memset(tri32[:], 1.0)
            nc.gpsimd.affine_select(
                out=tri32[:], in_=tri32[:],
                pattern=[[1, P]], compare_op=mybir.AluOpType.is_ge,
                fill=0.0, base=0, channel_multiplier=-1,
            )
            tri = pers.tile([P, P], MD, name="tri")
            nc.vector.tensor_copy(out=tri[:], in_=tri32[:])

            XT = [pers.tile([P, S], MD, name=f"xt{kt}") for kt in range(KT_N)]
            QT = [pers.tile([P, S], MD, name=f"qt{ft}") for ft in range(FT_N)]
            KTs = [pers.tile([P, S], MD, name=f"kt{ft}") for ft in range(FT_N)]
            VA = [pers.tile([P, H_LOC * (D + 1)], MD, name=f"va{rt}")
                  for rt in range(len(JB))]
            AOT = [pers.tile([P, S], MD, name=f"aot{ft}") for ft in range(FT_N)]

            # ---- V projection (natural layout, ones column appended) ----
            hpc = 512 // D  # heads per 512-wide f chunk
            fchunks = _chunks(E, 512)
            wv_tiles = [wvp.tile([P, KT_N, 512], MD, name=f"wv{fc}", tag=f"wv{fc}")
                        for fc in range(len(fchunks))]
            # interleave wv-slice and xT-tile loads so the first V matmuls
            # can issue as soon as (wv[:,0,:], xT[0]) land
            for kt in range(KT_N):
                for fc, (f0, fw) in enumerate(fchunks):
                    nc.sync.dma_start(
                        out=wv_tiles[fc][:, kt, :fw],
                        in_=_wslices(wvT.ap(), f0, fw)[:, kt, :])
                nc.sync.dma_start(out=XT[kt][:],
                                  in_=xT[kt * P:(kt + 1) * P, :])

            def emit_v(rts):
                for rt in rts:
                    if rt >= len(JB):
                        continue
                    r0, rsz = JB[rt]
                    for fc, (f0, fw) in enumerate(fchunks):
                        wt = wv_tiles[fc]
                        ps = psP.tile([P, 512], F32, name="pv", tag="pp", bufs=2)
                        for kt in range(KT_N):
                            nc.tensor.matmul(
                                ps[:rsz, :fw],
                                mm(XT[kt][:, r0:r0 + rsz]),
                                mm(wt[:, kt, :fw]),
                                start=(kt == 0), stop=(kt == KT_N - 1),
                            )
                        dst = VA[rt][:].rearrange("p (h c) -> p h c", c=D + 1)
                        nc.vector.tensor_copy(
                            out=dst[:rsz, fc * hpc:fc * hpc + fw // D, 0:D],
                            in_=ps[:rsz, :fw].rearrange("p (h d) -> p h d", d=D),
                        )
                    va3 = VA[rt][:].rearrange("p (h c) -> p h c", c=D + 1)
                    if MD == mybir.dt.float32r:
                        nc.gpsimd.memset(va3[:rsz, :, D:D + 1].bitcast(F32), 1.0)
                    else:
                        nc.gpsimd.memset(va3[:rsz, :, D:D + 1], 1.0)

            def proj_qk_gen(ft):
                for which, wdram, dst in (("q", wqT, QT), ("k", wkT, KTs)):
                    wt = wqkp.tile([P, KT_N, P], MD, name="wqk", tag="wqk")
                    nc.sync.dma_start(out=wt[:], in_=_wslices(wdram.ap(), ft * P, P))
                    for rc, (c0, cw) in enumerate(R_CH):
                        ps = psP.tile([P, 512], F32, name="pp", tag="pp", bufs=2)
                        for kt in range(KT_N):
                            nc.tensor.matmul(
                                ps[:, :cw],
                                mm(wt[:, kt, :]),
                                mm(XT[kt][:, c0:c0 + cw]),
                                start=(kt == 0), stop=(kt == KT_N - 1),
                            )
                        if which == "q":
                            nc.vector.tensor_scalar(
                                out=dst[ft][:, c0:c0 + cw], in0=ps[:, :cw],
                                scalar1=bq_sb[:, ft:ft + 1], scalar2=None,
                                op0=mybir.AluOpType.add,
                            )
                        else:
                            nc.vector.tensor_copy(
                                out=dst[ft][:, c0:c0 + cw], in_=ps[:, :cw])
                        yield

            def proj_qk(ft):
                for _ in proj_qk_gen(ft):
                    pass

            def attn_ft(ic, ft, mtiles, filler=None):
                c0, cw = R_CH[ic]
                nblk = (min(len(JB), (c0 + cw + P - 1) // P)
                        if causal else len(JB))
                pso = [psO.tile([D + 1, 512], F32, name=f"po{half}",
                                tag="po", bufs=2)
                       for half in range(2)]
                # diagonal-containing blocks first so the chunk-end attnV
                # gates on a short (non-masked) exp chain
                if causal:
                    cut = max(0, nblk - (cw + P - 1) // P)
                    order = list(range(cut, nblk)) + list(range(cut))
                else:
                    order = list(range(nblk))
                for n_i, jb in enumerate(order):
                    j0, jsz = JB[jb]
                    vo = max(0, j0 - c0) if causal else 0
                    # both halves' scores land in one 2-bank psum pair so a
                    # single ACTIVATE exps them together (halves ACT op count)
                    psp = psS.tile([P, 2, 512], F32, name="psp",
                                   tag="ps", bufs=2)
                    for half in range(2):
                        d0 = D * half
                        nc.tensor.matmul(
                            psp[:jsz, half, vo:cw],
                            mm(KTs[ft][d0:d0 + D, j0:j0 + jsz]),
                            mm(QT[ft][d0:d0 + D, c0 + vo:c0 + cw]),
                            start=True, stop=True,
                            tile_position=(d0, 0),
                        )
                    if not causal:
                        for half in range(2):
                            nc.vector.tensor_tensor(
                                out=psp[:jsz, half, :cw],
                                in0=psp[:jsz, half, :cw],
                                in1=mtiles[jb][:jsz, :cw],
                                op=mybir.AluOpType.add,
                            )
                    atp = apool.tile([P, 2, 512], MD, name="atp")
                    nc.scalar.activation(
                        out=atp[:jsz, :, vo:cw], in_=psp[:jsz, :, vo:cw],
                        func=mybir.ActivationFunctionType.Exp,
                    )
                    if causal and j0 >= c0:
                        # zero attn where j > i on the diagonal square
                        for half in range(2):
                            nc.vector.tensor_tensor(
                                out=atp[:jsz, half, vo:vo + jsz],
                                in0=atp[:jsz, half, vo:vo + jsz],
                                in1=tri[:jsz, :jsz],
                                op=mybir.AluOpType.mult,
                            )
                    va3 = VA[jb][:].rearrange("p (h c) -> p h c", c=D + 1)
                    for half in range(2):
                        nc.tensor.matmul(
                            pso[half][:, vo:cw],
                            mm(va3[:jsz, 2 * ft + half, :]),
                            mm(atp[:jsz, half, vo:cw]),
                            start=(n_i == 0), stop=(n_i == nblk - 1),
                        )
                    if filler is not None and n_i % 4 == 3:
                        filler()
                ssums = []
                for half in range(2):
                    ssum = spool.tile([1, 512], NRM, name=f"ssum{half}",
                                      tag="ssum")
                    nc.vector.tensor_copy(
                        out=ssum[:, :cw], in_=pso[half][D:D + 1, :cw])
                    ssums.append(ssum)
                for half in range(2):
                    d0 = D * half
                    psb = psP.tile([D, 512], F32, name="psb", tag="pp", bufs=2)
                    nc.tensor.matmul(
                        psb[:, :cw], mm(ones64[:, :]), mm(ssums[half][:, :cw]),
                        start=True, stop=True,
                    )
                    rb = spool.tile([D, 512], F32, name="rb")
                    nc.vector.reciprocal_approx_fast(
                        out=rb[:, :cw], in_=psb[:, :cw])
                    nc.vector.tensor_tensor(
                        out=AOT[ft][d0:d0 + D, c0:c0 + cw],
                        in0=pso[half][0:D, :cw], in1=rb[:, :cw],
                        op=mybir.AluOpType.mult,
                    )

            def emit_yt(ot, rc, wo_t):
                c0, cw = R_CH[rc]
                psy = psP.tile([P, 512], F32, name="py", tag="pp", bufs=2)
                for ft in range(FT_N):
                    nc.tensor.matmul(
                        psy[:, :cw],
                        mm(wo_t[:, ft, :]),
                        mm(AOT[ft][:, c0:c0 + cw]),
                        start=(ft == 0), stop=(ft == FT_N - 1),
                    )
                yt = evp.tile([P, 512], F32, name="yt", tag="yt")
                nc.vector.tensor_scalar(
                    out=yt[:, :cw], in0=psy[:, :cw],
                    scalar1=bo_sb[:, ot:ot + 1], scalar2=None,
                    op0=mybir.AluOpType.add,
                )
                nc.sync.dma_start(
                    out=yT[ot * P:(ot + 1) * P, c0:c0 + cw], in_=yt[:, :cw])

            if causal:
                wo_tiles = []
                for ot in range(FT_N):
                    wt = wop.tile([P, KT_N, P], MD, name=f"wo{ot}",
                                  tag=f"wo{ot}")
                    nc.sync.dma_start(out=wt[:],
                                      in_=_wslices(woT.ap(), ot * P, P))
                    wo_tiles.append(wt)
                nb0 = min(len(JB), (R_CH[0][0] + R_CH[0][1] + P - 1) // P)
                emit_v(range(nb0))
                proj_qk(0)
                nbp = nb0
                for ft in range(FT_N):
                    gen = proj_qk_gen(ft + 1) if ft + 1 < FT_N else None

                    def pump():
                        if gen is not None:
                            next(gen, None)

                    for ic in range(len(R_CH)):
                        attn_ft(ic, ft, None, filler=pump)
                        if ft == 0 and ic + 1 < len(R_CH):
                            c0n, cwn = R_CH[ic + 1]
                            nbn = min(len(JB), (c0n + cwn + P - 1) // P)
                            emit_v(range(nbp, nbn))
                            nbp = nbn
                        if ft == FT_N - 1:
                            # last ft has no next-ft projection filler: use the
                            # now-ready yT chunk as PE filler instead
                            for ot in range(FT_N):
                                emit_yt(ot, ic, wo_tiles[ot])
                    if gen is not None:
                        for _ in gen:
                            pass
            else:
                emit_v(range(len(JB)))
                for ft in range(FT_N):
                    proj_qk(ft)
                with tc.tile_pool(name="maskp", bufs=1) as mpool:
                    for ic, (c0, cw) in enumerate(R_CH):
                        mtiles = []
                        for jb, (j0, jsz) in enumerate(JB):
                            mt = mpool.tile([P, 512], F32, name=f"m{jb}")
                            nc.sync.dma_start(
                                out=mt[:jsz, :cw],
                                in_=maskT[j0:j0 + jsz, c0:c0 + cw])
                            mtiles.append(mt)
                        for ft in range(FT_N):
                            attn_ft(ic, ft, mtiles)
                for ot in range(FT_N):
                    wt = wop.tile([P, KT_N, P], MD, name=f"wo{ot}", tag="wo",
                                  bufs=2)
                    nc.sync.dma_start(out=wt[:], in_=_wslices(woT.ap(), ot * P, P))
                    for rc in range(len(R_CH)):
                        emit_yt(ot, rc, wt)

    nc.compile()
    return nc


_CACHE: dict = {}


def _get_nc(causal: bool):
    if causal not in _CACHE:
        _CACHE[causal] = build(causal)
    return _CACHE[causal]


def _is_causal(mask: np.ndarray) -> bool:
    if mask.shape != (S, S):
        return False
    expect = np.where(np.tril(np.ones((S, S), dtype=bool)), np.float32(0.0),
                      np.float32(NEG))
    return bool(np.array_equal(mask, expect))


MM_NP = ml_dtypes.bfloat16  # numpy dtype matching build()'s default mm_dt


def prep_inputs(x, mask, Wq, bq, Wk, Wv, bv, Wo, bo):
    """Host-side preprocessing shared by kernel() and the bench harness."""
    scale = np.float32(1.0 / np.sqrt(D))
    xT = np.ascontiguousarray(np.transpose(x, (0, 2, 1)).astype(np.float32)).astype(MM_NP)
    common = {
        "wqT": np.ascontiguousarray((Wq.astype(np.float32) * scale).T).astype(MM_NP),
        "wkT": np.ascontiguousarray(Wk.astype(np.float32).T).astype(MM_NP),
        "wvT": np.ascontiguousarray(Wv.astype(np.float32).T).astype(MM_NP),
        "woT": np.ascontiguousarray(Wo.astype(np.float32).T).astype(MM_NP),
        "bq": (bq.astype(np.float32) * scale),
        "bo": (bo.astype(np.float32) + Wo.astype(np.float32) @ bv.astype(np.float32)),
    }
    causal = _is_causal(np.asarray(mask))
    if not causal:
        common["maskT"] = np.ascontiguousarray(np.asarray(mask, np.float32).T)
    in_maps = [dict(common, xT=xT[b]) for b in range(B)]
    return causal, in_maps


_RUNNER: dict = {}


def _get_runner(causal: bool):
    """Compile once per mask-variant; cache the jitted SPMD executable."""
    if causal in _RUNNER:
        return _RUNNER[causal]
    import jax
    from jax.sharding import Mesh, PartitionSpec, NamedSharding
    import warnings
    with warnings.catch_warnings():
        warnings.simplefilter("ignore")
        from jax.experimental.shard_map import shard_map
    from concourse import bass2jax
    from concourse.bass2jax import _bass_exec_p, install_neuronx_cc_hook

    nc = _get_nc(causal)
    install_neuronx_cc_hook()
    partition_name = (nc.partition_id_tensor.name
                      if nc.partition_id_tensor else None)
    in_names, out_names, out_avals = [], [], []
    for alloc in nc.m.functions[0].allocations:
        if not isinstance(alloc, mybir.MemoryLocationSet):
            continue
        name = alloc.memorylocations[0].name
        if alloc.kind == "ExternalInput":
            if name != partition_name:
                in_names.append(name)
        elif alloc.kind == "ExternalOutput":
            out_names.append(name)
            out_avals.append(jax.core.ShapedArray(
                tuple(alloc.tensor_shape), mybir.dt.np(alloc.dtype)))
    n_params = len(in_names)
    n_outs = len(out_names)

    def _body(*args):
        operands = list(args)
        names = list(in_names) + list(out_names)
        if partition_name is not None:
            operands.append(bass2jax.partition_id_tensor())
            names.append(partition_name)
        outs = _bass_exec_p.bind(
            *operands,
            out_avals=tuple(out_avals),
            in_names=tuple(names),
            out_names=tuple(out_names),
            lowering_input_output_aliases=(),
            sim_require_finite=True,
            sim_require_nnan=True,
            nc=nc,
        )
        return tuple(outs)

    devices = jax.devices()[:B]
    mesh = Mesh(np.asarray(devices), ("core",))
    in_specs = (PartitionSpec("core"),) * (n_params + n_outs)
    out_specs = (PartitionSpec("core"),) * n_outs
    fn = jax.jit(
        shard_map(_body, mesh=mesh, in_specs=in_specs, out_specs=out_specs,
                  check_rep=False),
        donate_argnums=tuple(range(n_params, n_params + n_outs)),
        keep_unused=True,
    )
    runner = (fn, in_names, out_names, out_avals)
    _RUNNER[causal] = runner
    return runner


def kernel(x, mask, Wq, bq, Wk, Wv, bv, Wo, bo):
    causal, in_maps = prep_inputs(x, mask, Wq, bq, Wk, Wv, bv, Wo, bo)
    fn, in_names, out_names, out_avals = _get_runner(causal)
    cat = [np.concatenate([np.asarray(m[n]) for m in in_maps], axis=0)
           for n in in_names]
    zs = [np.zeros((B * a.shape[0], *a.shape[1:]), a.dtype) for a in out_avals]
    outs = fn(*cat, *zs)
    yT = np.asarray(outs[out_names.index("yT")]).reshape(B, E, S)
    out = np.ascontiguousarray(yT.transpose(0, 2, 1).astype(np.float32))
    return out



# revision 8
# speedup vs baseline: 262.9850x; 262.9850x over previous
"""Multi-head attention (B=8, S=1500, E=1024, H=16, D=64) on 8 trn2 NeuronCores.

Sharding: pure data-parallel over batch — core b computes batch element b
end-to-end (no collectives).

Causal fast path (the harness case) uses mixed fp8/fp16 precision:
  - Q/K/V/O projections run as fp8e4m3 DoubleRow matmuls (2 k-tiles per
    instruction -> 2x PE throughput vs bf16). Host pre-scales x by 16 and
    weights by 64 (power-of-2) so fp8 values sit in the normal range; the
    descales fold into the existing psum-eviction tensor_scalar ops.
  - scores / exp / attnV run in fp16 (same PE rate as bf16, 8x the mantissa).
  - hi-region: outputs at early rows are large (short causal span -> out ~ v_i)
    while later rows average hundreds of values; absmax error budget is set by
    those early rows. So q columns i<128, k/v positions j<128, and y columns
    i<128 are computed with fp16 weights end-to-end. CPU-emulated end-to-end
    absmax rel err of this exact config: 5.1e-3 (gate 2e-2).
  - softmax normalization rides the attnV matmul: V-tile gets a 65th column
    of 1/8 whose output row is the (scaled) attn row-sum; reciprocal times 8
    is folded so AO comes out x8, in fp8 range; Wo descale absorbs it.

General-mask fallback path is the previous all-bf16 kernel.
"""

import sys
import numpy as np
import ml_dtypes

for _p in ("/opt/trn_rl_repo",):
    if _p not in sys.path:
        sys.path.append(_p)

import concourse.bass as bass
import concourse.mybir as mybir
import concourse.tile as tile
from concourse import bacc
from concourse.bass_utils import run_bass_kernel_spmd

F32 = mybir.dt.float32
F16 = mybir.dt.float16
F8 = mybir.dt.float8e4

B, S, E, H, D = 8, 1500, 1024, 16, 64
P = 128
HI = 128          # hi-precision region width (rows/cols)
NEG = -1e9


def _chunks(total, step):
    return [(c0, min(step, total - c0)) for c0 in range(0, total, step)]


def _wslices(dram_ap, col0, cols):
    """[E, E] weight -> [P, E//P, cols] AP for a column slice (k on partition)."""
    return dram_ap.rearrange("(kt p) f -> p kt f", p=P)[:, :, col0:col0 + cols]


def build_causal():
    KT_N = E // P            # k-tiles over the embedding dim (8)
    FT_N = E // P            # f-tiles (8)
    TP_N = KT_N // 2         # DoubleRow k-tile pairs (4)
    R_CH = _chunks(S, 512)   # i/r chunks
    JB = _chunks(S, P)       # j blocks
    OCH = [(HI, 512 - HI)] + _chunks(S, 512)[1:]  # fp8 y chunks (cols >= HI)
    NRM = mybir.dt.float32r
    DR = mybir.MatmulPerfMode.DoubleRow
    nc = bacc.Bacc("TRN2", target_bir_lowering=False, debug=False,
                   num_devices=8)

    x8T = nc.dram_tensor("x8T", [E, S], F8, kind="ExternalInput")
    # x8 re-arranged [p, tp, jb, 2, 128] (pairs contiguous) for the V-proj
    # stationary operand: Ldweights requires contiguous DoubleRow pairs.
    NJB = (S + P - 1) // P
    x8v = nc.dram_tensor("x8v", [P, (KT_N // 2) * NJB * 2 * P], F8,
                         kind="ExternalInput")
    x16T = nc.dram_tensor("x16T", [E, HI], F16, kind="ExternalInput")
    wq8T = nc.dram_tensor("wq8T", [E, E], F8, kind="ExternalInput")
    wk8T = nc.dram_tensor("wk8T", [E, E], F8, kind="ExternalInput")
    wv8T = nc.dram_tensor("wv8T", [E, E], F8, kind="ExternalInput")
    wo8T = nc.dram_tensor("wo8T", [E, E], F8, kind="ExternalInput")
    wq16T = nc.dram_tensor("wq16T", [E, E], F16, kind="ExternalInput")
    wk16T = nc.dram_tensor("wk16T", [E, E], F16, kind="ExternalInput")
    wv16T = nc.dram_tensor("wv16T", [E, E], F16, kind="ExternalInput")
    wo16T = nc.dram_tensor("wo16T", [E, E], F16, kind="ExternalInput")
    bq = nc.dram_tensor("bq", [E], F32, kind="ExternalInput")   # bq / 8
    bo = nc.dram_tensor("bo", [E], F32, kind="ExternalInput")   # bo + Wo@bv
    yT = nc.dram_tensor("yT", [E, S], F32, kind="ExternalOutput")

    # psum descales (power-of-2, exact):
    #   fp8 q psum = (16 x)(64 w) = 1024 q ; want q/8 stored -> 2^-13
    #   fp8 k/v psum = 1024 k -> 2^-10
    #   hi  q psum = q -> 2^-3 ; hi k/v -> 1
    #   fp8 y psum = (8 ao)(64 wo) = 512 y -> 2^-9 ; hi y psum = 8 y -> 2^-3
    SQ8, SK8, SQH = 2.0 ** -13, 2.0 ** -10, 0.125
    SY8, SYH = 2.0 ** -9, 0.125

    nc._allow_low_precision_reason = "mixed fp8/fp16 matmul operand chain"
    with tile.TileContext(nc) as tc:
        with (
            tc.tile_pool(name="persist", bufs=1) as pers,
            tc.tile_pool(name="wqk8p", bufs=2) as wqk8p,
            tc.tile_pool(name="wqk16p", bufs=2) as wqk16p,
            tc.tile_pool(name="wvp", bufs=1) as wvp,
            tc.tile_pool(name="wop", bufs=1) as wop,
            tc.tile_pool(name="attn", bufs=3) as apool,
            tc.tile_pool(name="small", bufs=3) as spool,
            tc.tile_pool(name="evp", bufs=3) as evp,
            tc.tile_pool(name="psP", bufs=1, space="PSUM") as psP,
            tc.tile_pool(name="psS", bufs=1, space="PSUM") as psS,
            tc.tile_pool(name="psO", bufs=1, space="PSUM") as psO,
        ):
            ones64 = pers.tile([1, D], NRM, name="ones64")
            nc.vector.memset(ones64[:].bitcast(F32), 1.0)
            bq_sb = pers.tile([P, FT_N], F32, name="bq_sb")
            nc.sync.dma_start(out=bq_sb[:], in_=bq.ap().rearrange("(t p) -> p t", p=P))
            bo_sb = pers.tile([P, FT_N], F32, name="bo_sb")
            nc.sync.dma_start(out=bo_sb[:], in_=bo.ap().rearrange("(t p) -> p t", p=P))

            # upper-triangular (incl diag) 0/1 mask for diagonal attn blocks
            tri32 = pers.tile([P, P], F32, name="tri32")
            nc.gpsimd.memset(tri32[:], 1.0)
            nc.gpsimd.affine_select(
                out=tri32[:], in_=tri32[:],
                pattern=[[1, P]], compare_op=mybir.AluOpType.is_ge,
                fill=0.0, base=0, channel_multiplier=-1,
            )
            tri = pers.tile([P, P], F16, name="tri")
            nc.vector.tensor_copy(out=tri[:], in_=tri32[:])

            XT8 = pers.tile([P, KT_N, S], F8, name="xt8")
            XV8 = pers.tile([P, TP_N, len(JB), 2, P], F8, name="xv8")
            XT16 = pers.tile([P, KT_N, HI], F16, name="xt16")
            QT = [pers.tile([P, S], F16, name=f"qt{ft}") for ft in range(FT_N)]
            KTs = [pers.tile([P, S], F16, name=f"kt{ft}") for ft in range(FT_N)]
            VA = [pers.tile([P, H * (D + 1)], F16, name=f"va{rt}")
                  for rt in range(len(JB))]
            AOT8 = pers.tile([P, FT_N, S], F8, name="aot8")
            AOT16 = pers.tile([P, FT_N, HI], F16, name="aot16")

            # ---- V projection weights + x loads (interleaved) ----
            hpc = 512 // D  # heads per 512-wide f chunk
            fchunks = _chunks(E, 512)
            wv8_t = [wvp.tile([P, KT_N, 512], F8, name=f"wv8{fc}",
                              tag=f"wv8{fc}") for fc in range(len(fchunks))]
            xv8_src = x8v.ap().rearrange("p (a b c d) -> p a b c d",
                                         a=TP_N, b=len(JB), c=2)
            for jb in (1, 2, 3):
                nc.sync.dma_start(out=XV8[:, :, jb, :, :],
                                  in_=xv8_src[:, :, jb, :, :])
            for kt in range(KT_N):
                for fc, (f0, fw) in enumerate(fchunks):
                    nc.sync.dma_start(
                        out=wv8_t[fc][:, kt, :fw],
                        in_=_wslices(wv8T.ap(), f0, fw)[:, kt, :])
                nc.sync.dma_start(out=XT8[:, kt, :], in_=x8T[kt * P:(kt + 1) * P, :])
            for jb in range(4, len(JB)):
                nc.sync.dma_start(out=XV8[:, :, jb, :, :],
                                  in_=xv8_src[:, :, jb, :, :])
            nc.sync.dma_start(
                out=XT16[:], in_=x16T.ap().rearrange("(kt p) s -> p kt s", p=P))
            wv16_t = [wvp.tile([P, KT_N, 512], F16, name=f"wv16{fc}",
                               tag=f"wv16{fc}") for fc in range(len(fchunks))]
            for fc, (f0, fw) in enumerate(fchunks):
                nc.sync.dma_start(out=wv16_t[fc][:],
                                  in_=_wslices(wv16T.ap(), f0, fw))

            def emit_v8(rts):
                """fp8 V projection for j-blocks >= 1."""
                for rt in rts:
                    if rt >= len(JB) or rt == 0:
                        continue
                    r0, rsz = JB[rt]
                    for fc, (f0, fw) in enumerate(fchunks):
                        ps = psP.tile([P, 512], F32, name="pv", tag="pp", bufs=2)
                        for tp in range(TP_N):
                            nc.tensor.matmul(
                                ps[:, :fw],
                                XV8[:, tp, rt, :, :],
                                wv8_t[fc][:, 2 * tp:2 * tp + 2, :fw],
                                start=(tp == 0), stop=(tp == TP_N - 1),
                                perf_mode=DR,
                            )
                        dst = VA[rt][:].rearrange("p (h c) -> p h c", c=D + 1)
                        nc.vector.tensor_scalar(
                            out=dst[:rsz, fc * hpc:fc * hpc + fw // D, 0:D],
                            in0=ps[:rsz, :fw].rearrange("p (h d) -> p h d", d=D),
                            scalar1=SK8, scalar2=None, op0=mybir.AluOpType.mult,
                        )
                    va3 = VA[rt][:].rearrange("p (h c) -> p h c", c=D + 1)
                    nc.gpsimd.memset(va3[:rsz, :, D:D + 1], 0.125)

            def emit_v_hi():
                """fp16 V projection for j-block 0."""
                rsz = P
                for fc, (f0, fw) in enumerate(fchunks):
                    ps = psP.tile([P, 512], F32, name="pv", tag="pp", bufs=2)
                    for kt in range(KT_N):
                        nc.tensor.matmul(
                            ps[:rsz, :fw],
                            XT16[:, kt, :],
                            wv16_t[fc][:, kt, :fw],
                            start=(kt == 0), stop=(kt == KT_N - 1),
                        )
                    dst = VA[0][:].rearrange("p (h c) -> p h c", c=D + 1)
                    nc.vector.tensor_copy(
                        out=dst[:rsz, fc * hpc:fc * hpc + fw // D, 0:D],
                        in_=ps[:rsz, :fw].rearrange("p (h d) -> p h d", d=D),
                    )
                va3 = VA[0][:].rearrange("p (h c) -> p h c", c=D + 1)
                nc.gpsimd.memset(va3[:rsz, :, D:D + 1], 0.125)

            def proj_qk_gen(ft):
                for which, w8d, w16d, dst in (("q", wq8T, wq16T, QT),
                                              ("k", wk8T, wk16T, KTs)):
                    wt8 = wqk8p.tile([P, KT_N, P], F8, name="wqk8", tag="wqk8")
                    nc.sync.dma_start(out=wt8[:], in_=_wslices(w8d.ap(), ft * P, P))
                    wt16 = wqk16p.tile([P, KT_N, P], F16, name="wqk16",
                                       tag="wqk16")
                    nc.sync.dma_start(out=wt16[:], in_=_wslices(w16d.ap(), ft * P, P))
                    for rc, (c0, cw) in enumerate(R_CH):
                        ps = psP.tile([P, 512], F32, name="pp", tag="pp", bufs=2)
                        for tp in range(TP_N):
                            nc.tensor.matmul(
                                ps[:, :cw],
                                wt8[:, 2 * tp:2 * tp + 2, :],
                                XT8[:, 2 * tp:2 * tp + 2, c0:c0 + cw],
                                start=(tp == 0), stop=(tp == TP_N - 1),
                                perf_mode=DR,
                            )
                        lo = HI if rc == 0 else 0  # cols < HI come from hi path
                        if which == "q":
                            nc.vector.tensor_scalar(
                                out=dst[ft][:, c0 + lo:c0 + cw],
                                in0=ps[:, lo:cw],
                                scalar1=SQ8, scalar2=bq_sb[:, ft:ft + 1],
                                op0=mybir.AluOpType.mult,
                                op1=mybir.AluOpType.add,
                            )
                        else:
                            nc.vector.tensor_scalar(
                                out=dst[ft][:, c0 + lo:c0 + cw],
                                in0=ps[:, lo:cw],
                                scalar1=SK8, scalar2=None,
                                op0=mybir.AluOpType.mult,
                            )
                        yield
                    # hi group: cols [0, HI) with fp16 operands
                    ps = psP.tile([P, 512], F32, name="pp", tag="pp", bufs=2)
                    for kt in range(KT_N):
                        nc.tensor.matmul(
                            ps[:, :HI],
                            wt16[:, kt, :],
                            XT16[:, kt, :],
                            start=(kt == 0), stop=(kt == KT_N - 1),
                        )
                    if which == "q":
                        nc.vector.tensor_scalar(
                            out=dst[ft][:, 0:HI], in0=ps[:, :HI],
                            scalar1=SQH, scalar2=bq_sb[:, ft:ft + 1],
                            op0=mybir.AluOpType.mult, op1=mybir.AluOpType.add,
                        )
                    else:
                        nc.vector.tensor_copy(out=dst[ft][:, 0:HI],
                                              in_=ps[:, :HI])
                    yield

            def proj_qk(ft):
                for _ in proj_qk_gen(ft):
                    pass

            def attn_ft(ic, ft, filler=None):
                c0, cw = R_CH[ic]
                nblk = min(len(JB), (c0 + cw + P - 1) // P)
                pso = [psO.tile([D + 1, 512], F32, name=f"po{half}",
                                tag="po", bufs=2)
                       for half in range(2)]
                # diagonal-containing blocks first so the chunk-end attnV
                # gates on a short (non-masked) exp chain
                cut = max(0, nblk - (cw + P - 1) // P)
                order = list(range(cut, nblk)) + list(range(cut))
                for n_i, jb in enumerate(order):
                    j0, jsz = JB[jb]
                    vo = max(0, j0 - c0)
                    # both halves' scores land in one 2-bank psum pair so a
                    # single ACTIVATE exps them together (halves ACT op count)
                    psp = psS.tile([P, 2, 512], F32, name="psp",
                                   tag="ps", bufs=2)
                    for half in range(2):
                        d0 = D * half
                        nc.tensor.matmul(
                            psp[:jsz, half, vo:cw],
                            KTs[ft][d0:d0 + D, j0:j0 + jsz],
                            QT[ft][d0:d0 + D, c0 + vo:c0 + cw],
                            start=True, stop=True,
                            tile_position=(d0, 0),
                        )
                    atp = apool.tile([P, 2, 512], F16, name="atp")
                    nc.scalar.activation(
                        out=atp[:jsz, :, vo:cw], in_=psp[:jsz, :, vo:cw],
                        func=mybir.ActivationFunctionType.Exp,
                    )
                    if j0 >= c0:
                        # zero attn where j > i on the diagonal square
                        for half in range(2):
                            nc.vector.tensor_tensor(
                                out=atp[:jsz, half, vo:vo + jsz],
                                in0=atp[:jsz, half, vo:vo + jsz],
                                in1=tri[:jsz, :jsz],
                                op=mybir.AluOpType.mult,
                            )
                    va3 = VA[jb][:].rearrange("p (h c) -> p h c", c=D + 1)
                    for half in range(2):
                        nc.tensor.matmul(
                            pso[half][:, vo:cw],
                            va3[:jsz, 2 * ft + half, :],
                            atp[:jsz, half, vo:cw],
                            start=(n_i == 0), stop=(n_i == nblk - 1),
                        )
                    if filler is not None and n_i % 4 == 3:
                        filler()
                ssums = []
                for half in range(2):
                    ssum = spool.tile([1, 512], NRM, name=f"ssum{half}",
                                      tag="ssum")
                    nc.vector.tensor_copy(
                        out=ssum[:, :cw], in_=pso[half][D:D + 1, :cw])
                    ssums.append(ssum)
                for half in range(2):
                    d0 = D * half
                    psb = psP.tile([D, 512], F32, name="psb", tag="pp", bufs=2)
                    nc.tensor.matmul(
                        psb[:, :cw], ones64[:, :], ssums[half][:, :cw],
                        start=True, stop=True,
                    )
                    rb = spool.tile([D, 512], F32, name="rb")
                    nc.vector.reciprocal_approx_fast(
                        out=rb[:, :cw], in_=psb[:, :cw])
                    # AO x8 (ones col = 1/8 makes rb = 8/sum)
                    nc.vector.tensor_tensor(
                        out=AOT8[d0:d0 + D, ft, c0:c0 + cw],
                        in0=pso[half][0:D, :cw], in1=rb[:, :cw],
                        op=mybir.AluOpType.mult,
                    )
                    if ic == 0:
                        nc.vector.tensor_tensor(
                            out=AOT16[d0:d0 + D, ft, 0:HI],
                            in0=pso[half][0:D, 0:HI], in1=rb[:, 0:HI],
                            op=mybir.AluOpType.mult,
                        )

            def emit_yt8(ot, oc, wo_t):
                c0, cw = OCH[oc]
                psy = psP.tile([P, 512], F32, name="py", tag="pp", bufs=2)
                for tp in range(TP_N):
                    nc.tensor.matmul(
                        psy[:, :cw],
                        wo_t[:, 2 * tp:2 * tp + 2, :],
                        AOT8[:, 2 * tp:2 * tp + 2, c0:c0 + cw],
                        start=(tp == 0), stop=(tp == TP_N - 1),
                        perf_mode=DR,
                    )
                yt = evp.tile([P, 512], F32, name="yt", tag="yt")
                nc.vector.tensor_scalar(
                    out=yt[:, :cw], in0=psy[:, :cw],
                    scalar1=SY8, scalar2=bo_sb[:, ot:ot + 1],
                    op0=mybir.AluOpType.mult, op1=mybir.AluOpType.add,
                )
                nc.sync.dma_start(
                    out=yT[ot * P:(ot + 1) * P, c0:c0 + cw], in_=yt[:, :cw])

            def emit_yt_hi(ot, wo16_t):
                psy = psP.tile([P, 512], F32, name="py", tag="pp", bufs=2)
                for ft in range(FT_N):
                    nc.tensor.matmul(
                        psy[:, :HI],
                        wo16_t[:, ft, :],
                        AOT16[:, ft, :],
                        start=(ft == 0), stop=(ft == FT_N - 1),
                    )
                yt = evp.tile([P, 512], F32, name="yt", tag="yt")
                nc.vector.tensor_scalar(
                    out=yt[:, :HI], in0=psy[:, :HI],
                    scalar1=SYH, scalar2=bo_sb[:, ot:ot + 1],
                    op0=mybir.AluOpType.mult, op1=mybir.AluOpType.add,
                )
                nc.sync.dma_start(out=yT[ot * P:(ot + 1) * P, 0:HI],
                                  in_=yt[:, :HI])

            # ---- schedule ----
            nb0 = min(len(JB), (R_CH[0][0] + R_CH[0][1] + P - 1) // P)
            emit_v8(range(1, nb0))
            emit_v_hi()
            proj_qk(0)
            # wo loads issued after the hot startup path
            wo8_tiles = []
            for ot in range(FT_N):
                wt = wop.tile([P, KT_N, P], F8, name=f"wo8{ot}", tag=f"wo8{ot}")
                nc.sync.dma_start(out=wt[:], in_=_wslices(wo8T.ap(), ot * P, P))
                wo8_tiles.append(wt)
            wo16_tiles = []
            for ot in range(FT_N):
                wt = wop.tile([P, KT_N, P], F16, name=f"wo16{ot}",
                              tag=f"wo16{ot}")
                nc.sync.dma_start(out=wt[:], in_=_wslices(wo16T.ap(), ot * P, P))
                wo16_tiles.append(wt)
            nbp = nb0
            for ft in range(FT_N):
                gen = proj_qk_gen(ft + 1) if ft + 1 < FT_N else None

                def pump():
                    if gen is not None:
                        next(gen, None)

                for ic in range(len(R_CH)):
                    attn_ft(ic, ft, filler=pump)
                    if ft == 0 and ic + 1 < len(R_CH):
                        c0n, cwn = R_CH[ic + 1]
                        nbn = min(len(JB), (c0n + cwn + P - 1) // P)
                        emit_v8(range(nbp, nbn))
                        nbp = nbn
                    if ft == FT_N - 1:
                        # last ft has no next-ft projection filler: use the
                        # now-ready yT chunk as PE filler instead
                        for ot in range(FT_N):
                            if ic == 0:
                                emit_yt_hi(ot, wo16_tiles[ot])
                            emit_yt8(ot, ic, wo8_tiles[ot])
                if gen is not None:
                    for _ in gen:
                        pass

    nc.compile()
    return nc


def build_general(mm_dt=mybir.dt.bfloat16):
    """All-bf16 fallback for a non-causal additive mask (not the harness
    case). Same as the previous baseline kernel's general path."""
    KT_N = E // P
    FT_N = E // P
    R_CH = _chunks(S, 512)
    JB = _chunks(S, P)
    nc = bacc.Bacc("TRN2", target_bir_lowering=False, debug=False,
                   num_devices=8)
    MD = mm_dt
    NRM = mybir.dt.float32r

    xT = nc.dram_tensor("xT", [E, S], MD, kind="ExternalInput")
    wqT = nc.dram_tensor("wqT", [E, E], MD, kind="ExternalInput")
    wkT = nc.dram_tensor("wkT", [E, E], MD, kind="ExternalInput")
    wvT = nc.dram_tensor("wvT", [E, E], MD, kind="ExternalInput")
    woT = nc.dram_tensor("woT", [E, E], MD, kind="ExternalInput")
    bq = nc.dram_tensor("bq", [E], F32, kind="ExternalInput")
    bo = nc.dram_tensor("bo", [E], F32, kind="ExternalInput")
    maskT = nc.dram_tensor("maskT", [S, S], F32, kind="ExternalInput")
    yT = nc.dram_tensor("yT", [E, S], F32, kind="ExternalOutput")

    nc._allow_low_precision_reason = "low-precision matmul operand chain"
    with tile.TileContext(nc) as tc:
        with (
            tc.tile_pool(name="persist", bufs=1) as pers,
            tc.tile_pool(name="wqkp", bufs=2) as wqkp,
            tc.tile_pool(name="wvp", bufs=1) as wvp,
            tc.tile_pool(name="wop", bufs=1) as wop,
            tc.tile_pool(name="attn", bufs=3) as apool,
            tc.tile_pool(name="small", bufs=3) as spool,
            tc.tile_pool(name="evp", bufs=3) as evp,
            tc.tile_pool(name="psP", bufs=1, space="PSUM") as psP,
            tc.tile_pool(name="psS", bufs=1, space="PSUM") as psS,
            tc.tile_pool(name="psO", bufs=1, space="PSUM") as psO,
        ):
            ones64 = pers.tile([1, D], NRM, name="ones64")
            nc.vector.memset(ones64[:].bitcast(F32), 1.0)
            bq_sb = pers.tile([P, FT_N], F32, name="bq_sb")
            nc.sync.dma_start(out=bq_sb[:], in_=bq.ap().rearrange("(t p) -> p t", p=P))
            bo_sb = pers.tile([P, FT_N], F32, name="bo_sb")
            nc.sync.dma_start(out=bo_sb[:], in_=bo.ap().rearrange("(t p) -> p t", p=P))

            XT = [pers.tile([P, S], MD, name=f"xt{kt}") for kt in range(KT_N)]
            QT = [pers.tile([P, S], MD, name=f"qt{ft}") for ft in range(FT_N)]
            KTs = [pers.tile([P, S], MD, name=f"kt{ft}") for ft in range(FT_N)]
            VA = [pers.tile([P, H * (D + 1)], MD, name=f"va{rt}")
                  for rt in range(len(JB))]
            AOT = [pers.tile([P, S], MD, name=f"aot{ft}") for ft in range(FT_N)]

            hpc = 512 // D
            fchunks = _chunks(E, 512)
            wv_tiles = [wvp.tile([P, KT_N, 512], MD, name=f"wv{fc}", tag=f"wv{fc}")
                        for fc in range(len(fchunks))]
            for kt in range(KT_N):
                for fc, (f0, fw) in enumerate(fchunks):
                    nc.sync.dma_start(
                        out=wv_tiles[fc][:, kt, :fw],
                        in_=_wslices(wvT.ap(), f0, fw)[:, kt, :])
                nc.sync.dma_start(out=XT[kt][:],
                                  in_=xT[kt * P:(kt + 1) * P, :])

            def emit_v(rts):
                for rt in rts:
                    if rt >= len(JB):
                        continue
                    r0, rsz = JB[rt]
                    for fc, (f0, fw) in enumerate(fchunks):
                        wt = wv_tiles[fc]
                        ps = psP.tile([P, 512], F32, name="pv", tag="pp", bufs=2)
                        for kt in range(KT_N):
                            nc.tensor.matmul(
                                ps[:rsz, :fw],
                                XT[kt][:, r0:r0 + rsz],
                                wt[:, kt, :fw],
                                start=(kt == 0), stop=(kt == KT_N - 1),
                            )
                        dst = VA[rt][:].rearrange("p (h c) -> p h c", c=D + 1)
                        nc.vector.tensor_copy(
                            out=dst[:rsz, fc * hpc:fc * hpc + fw // D, 0:D],
                            in_=ps[:rsz, :fw].rearrange("p (h d) -> p h d", d=D),
                        )
                    va3 = VA[rt][:].rearrange("p (h c) -> p h c", c=D + 1)
                    nc.gpsimd.memset(va3[:rsz, :, D:D + 1], 1.0)

            def proj_qk(ft):
                for which, wdram, dst in (("q", wqT, QT), ("k", wkT, KTs)):
                    wt = wqkp.tile([P, KT_N, P], MD, name="wqk", tag="wqk")
                    nc.sync.dma_start(out=wt[:], in_=_wslices(wdram.ap(), ft * P, P))
                    for rc, (c0, cw) in enumerate(R_CH):
                        ps = psP.tile([P, 512], F32, name="pp", tag="pp", bufs=2)
                        for kt in range(KT_N):
                            nc.tensor.matmul(
                                ps[:, :cw],
                                wt[:, kt, :],
                                XT[kt][:, c0:c0 + cw],
                                start=(kt == 0), stop=(kt == KT_N - 1),
                            )
                        if which == "q":
                            nc.vector.tensor_scalar(
                                out=dst[ft][:, c0:c0 + cw], in0=ps[:, :cw],
                                scalar1=bq_sb[:, ft:ft + 1], scalar2=None,
                                op0=mybir.AluOpType.add,
                            )
                        else:
                            nc.vector.tensor_copy(
                                out=dst[ft][:, c0:c0 + cw], in_=ps[:, :cw])

            def attn_ft(ic, ft, mtiles):
                c0, cw = R_CH[ic]
                nblk = len(JB)
                pso = [psO.tile([D + 1, 512], F32, name=f"po{half}",
                                tag="po", bufs=2)
                       for half in range(2)]
                for n_i, jb in enumerate(range(nblk)):
                    j0, jsz = JB[jb]
                    psp = psS.tile([P, 2, 512], F32, name="psp",
                                   tag="ps", bufs=2)
                    for half in range(2):
                        d0 = D * half
                        nc.tensor.matmul(
                            psp[:jsz, half, :cw],
                            KTs[ft][d0:d0 + D, j0:j0 + jsz],
                            QT[ft][d0:d0 + D, c0:c0 + cw],
                            start=True, stop=True,
                            tile_position=(d0, 0),
                        )
                    for half in range(2):
                        nc.vector.tensor_tensor(
                            out=psp[:jsz, half, :cw],
                            in0=psp[:jsz, half, :cw],
                            in1=mtiles[jb][:jsz, :cw],
                            op=mybir.AluOpType.add,
                        )
                    atp = apool.tile([P, 2, 512], MD, name="atp")
                    nc.scalar.activation(
                        out=atp[:jsz, :, :cw], in_=psp[:jsz, :, :cw],
                        func=mybir.ActivationFunctionType.Exp,
                    )
                    va3 = VA[jb][:].rearrange("p (h c) -> p h c", c=D + 1)
                    for half in range(2):
                        nc.tensor.matmul(
                            pso[half][:, :cw],
                            va3[:jsz, 2 * ft + half, :],
                            atp[:jsz, half, :cw],
                            start=(n_i == 0), stop=(n_i == nblk - 1),
                        )
                ssums = []
                for half in range(2):
                    ssum = spool.tile([1, 512], NRM, name=f"ssum{half}",
                                      tag="ssum")
                    nc.vector.tensor_copy(
                        out=ssum[:, :cw], in_=pso[half][D:D + 1, :cw])
                    ssums.append(ssum)
                for half in range(2):
                    d0 = D * half
                    psb = psP.tile([D, 512], F32, name="psb", tag="pp", bufs=2)
                    nc.tensor.matmul(
                        psb[:, :cw], ones64[:, :], ssums[half][:, :cw],
                        start=True, stop=True,
                    )
                    rb = spool.tile([D, 512], F32, name="rb")
                    nc.vector.reciprocal_approx_fast(
                        out=rb[:, :cw], in_=psb[:, :cw])
                    nc.vector.tensor_tensor(
                        out=AOT[ft][d0:d0 + D, c0:c0 + cw],
                        in0=pso[half][0:D, :cw], in1=rb[:, :cw],
                        op=mybir.AluOpType.mult,
                    )

            def emit_yt(ot, rc, wo_t):
                c0, cw = R_CH[rc]
                psy = psP.tile([P, 512], F32, name="py", tag="pp", bufs=2)
                for ft in range(FT_N):
                    nc.tensor.matmul(
                        psy[:, :cw],
                        wo_t[:, ft, :],
                        AOT[ft][:, c0:c0 + cw],
                        start=(ft == 0), stop=(ft == FT_N - 1),
                    )
                yt = evp.tile([P, 512], F32, name="yt", tag="yt")
                nc.vector.tensor_scalar(
                    out=yt[:, :cw], in0=psy[:, :cw],
                    scalar1=bo_sb[:, ot:ot + 1], scalar2=None,
                    op0=mybir.AluOpType.add,
                )
                nc.sync.dma_start(
                    out=yT[ot * P:(ot + 1) * P, c0:c0 + cw], in_=yt[:, :cw])

            emit_v(range(len(JB)))
            for ft in range(FT_N):
                proj_qk(ft)
            with tc.tile_pool(name="maskp", bufs=1) as mpool:
                for ic, (c0, cw) in enumerate(R_CH):
                    mtiles = []
                    for jb, (j0, jsz) in enumerate(JB):
                        mt = mpool.tile([P, 512], F32, name=f"m{jb}")
                        nc.sync.dma_start(
                            out=mt[:jsz, :cw],
                            in_=maskT[j0:j0 + jsz, c0:c0 + cw])
                        mtiles.append(mt)
                    for ft in range(FT_N):
                        attn_ft(ic, ft, mtiles)
            for ot in range(FT_N):
                wt = wop.tile([P, KT_N, P], MD, name=f"wo{ot}", tag="wo",
                              bufs=2)
                nc.sync.dma_start(out=wt[:], in_=_wslices(woT.ap(), ot * P, P))
                for rc in range(len(R_CH)):
                    emit_yt(ot, rc, wt)

    nc.compile()
    return nc


_CACHE: dict = {}


def _get_nc(causal: bool):
    if causal not in _CACHE:
        _CACHE[causal] = build_causal() if causal else build_general()
    return _CACHE[causal]


def _is_causal(mask: np.ndarray) -> bool:
    if mask.shape != (S, S):
        return False
    expect = np.where(np.tril(np.ones((S, S), dtype=bool)), np.float32(0.0),
                      np.float32(NEG))
    return bool(np.array_equal(mask, expect))


F8_NP = ml_dtypes.float8_e4m3
F16_NP = np.float16
BF16_NP = ml_dtypes.bfloat16


def _f8(a, s):
    return np.clip(np.asarray(a, np.float32) * np.float32(s),
                   -240.0, 240.0).astype(F8_NP)


def _x8v(x8):
    """[E, S] fp8 -> [P, tp*jb*2*128] with DoubleRow pairs contiguous."""
    njb = (S + P - 1) // P
    xp = np.zeros((E, njb * P), F8_NP)
    xp[:, :S] = x8
    v = xp.reshape(4, 2, P, njb, P)   # (tp, k2, p, jb, c)
    v = v.transpose(2, 0, 3, 1, 4)    # (p, tp, jb, k2, c)
    return np.ascontiguousarray(v.reshape(P, -1))


def prep_inputs(x, mask, Wq, bq, Wk, Wv, bv, Wo, bo):
    """Host-side preprocessing shared by kernel() and the bench harness."""
    scale = np.float32(1.0 / np.sqrt(D))
    causal = _is_causal(np.asarray(mask))
    if causal:
        xT = np.ascontiguousarray(
            np.transpose(np.asarray(x, np.float32), (0, 2, 1)))
        common = {
            "wq8T": np.ascontiguousarray(_f8(Wq.T, 64.0)),
            "wk8T": np.ascontiguousarray(_f8(Wk.T, 64.0)),
            "wv8T": np.ascontiguousarray(_f8(Wv.T, 64.0)),
            "wo8T": np.ascontiguousarray(_f8(Wo.T, 64.0)),
            "wq16T": np.ascontiguousarray(np.asarray(Wq.T, np.float32)).astype(F16_NP),
            "wk16T": np.ascontiguousarray(np.asarray(Wk.T, np.float32)).astype(F16_NP),
            "wv16T": np.ascontiguousarray(np.asarray(Wv.T, np.float32)).astype(F16_NP),
            "wo16T": np.ascontiguousarray(np.asarray(Wo.T, np.float32)).astype(F16_NP),
            "bq": (np.asarray(bq, np.float32) * scale),
            "bo": (np.asarray(bo, np.float32)
                   + np.asarray(Wo, np.float32) @ np.asarray(bv, np.float32)),
        }
        in_maps = []
        for b in range(B):
            x8 = _f8(xT[b], 16.0)
            in_maps.append(dict(
                common, x8T=x8, x8v=_x8v(x8),
                x16T=np.ascontiguousarray(xT[b][:, :HI]).astype(F16_NP)))
        return causal, in_maps
    xTb = np.ascontiguousarray(
        np.transpose(np.asarray(x, np.float32), (0, 2, 1))).astype(BF16_NP)
    common = {
        "wqT": np.ascontiguousarray((np.asarray(Wq, np.float32) * scale).T).astype(BF16_NP),
        "wkT": np.ascontiguousarray(np.asarray(Wk, np.float32).T).astype(BF16_NP),
        "wvT": np.ascontiguousarray(np.asarray(Wv, np.float32).T).astype(BF16_NP),
        "woT": np.ascontiguousarray(np.asarray(Wo, np.float32).T).astype(BF16_NP),
        "bq": (np.asarray(bq, np.float32) * scale),
        "bo": (np.asarray(bo, np.float32)
               + np.asarray(Wo, np.float32) @ np.asarray(bv, np.float32)),
        "maskT": np.ascontiguousarray(np.asarray(mask, np.float32).T),
    }
    in_maps = [dict(common, xT=xTb[b]) for b in range(B)]
    return causal, in_maps


_RUNNER: dict = {}


def _get_runner(causal: bool):
    """Compile once per mask-variant; cache the jitted SPMD executable."""
    if causal in _RUNNER:
        return _RUNNER[causal]
    import jax
    from jax.sharding import Mesh, PartitionSpec, NamedSharding
    import warnings
    with warnings.catch_warnings():
        warnings.simplefilter("ignore")
        from jax.experimental.shard_map import shard_map
    from concourse import bass2jax
    from concourse.bass2jax import _bass_exec_p, install_neuronx_cc_hook

    nc = _get_nc(causal)
    install_neuronx_cc_hook()
    partition_name = (nc.partition_id_tensor.name
                      if nc.partition_id_tensor else None)
    in_names, out_names, out_avals = [], [], []
    for alloc in nc.m.functions[0].allocations:
        if not isinstance(alloc, mybir.MemoryLocationSet):
            continue
        name = alloc.memorylocations[0].name
        if alloc.kind == "ExternalInput":
            if name != partition_name:
                in_names.append(name)
        elif alloc.kind == "ExternalOutput":
            out_names.append(name)
            out_avals.append(jax.core.ShapedArray(
                tuple(alloc.tensor_shape), mybir.dt.np(alloc.dtype)))
    n_params = len(in_names)
    n_outs = len(out_names)

    def _body(*args):
        operands = list(args)
        names = list(in_names) + list(out_names)
        if partition_name is not None:
            operands.append(bass2jax.partition_id_tensor())
            names.append(partition_name)
        outs = _bass_exec_p.bind(
            *operands,
            out_avals=tuple(out_avals),
            in_names=tuple(names),
            out_names=tuple(out_names),
            lowering_input_output_aliases=(),
            sim_require_finite=True,
            sim_require_nnan=True,
            nc=nc,
        )
        return tuple(outs)

    devices = jax.devices()[:B]
    mesh = Mesh(np.asarray(devices), ("core",))
    in_specs = (PartitionSpec("core"),) * (n_params + n_outs)
    out_specs = (PartitionSpec("core"),) * n_outs
    fn = jax.jit(
        shard_map(_body, mesh=mesh, in_specs=in_specs, out_specs=out_specs,
                  check_rep=False),
        donate_argnums=tuple(range(n_params, n_params + n_outs)),
        keep_unused=True,
    )
    runner = (fn, in_names, out_names, out_avals)
    _RUNNER[causal] = runner
    return runner


def kernel(x, mask, Wq, bq, Wk, Wv, bv, Wo, bo):
    causal, in_maps = prep_inputs(x, mask, Wq, bq, Wk, Wv, bv, Wo, bo)
    fn, in_names, out_names, out_avals = _get_runner(causal)
    cat = [np.concatenate([np.asarray(m[n]) for m in in_maps], axis=0)
           for n in in_names]
    zs = [np.zeros((B * a.shape[0], *a.shape[1:]), a.dtype) for a in out_avals]
    outs = fn(*cat, *zs)
    yT = np.asarray(outs[out_names.index("yT")]).reshape(B, E, S)
    out = np.ascontiguousarray(yT.transpose(0, 2, 1).astype(np.float32))
    return out


# revision 14
# speedup vs baseline: 276.5837x; 1.0517x over previous
"""Multi-head attention (B=8, S=1500, E=1024, H=16, D=64) on 8 trn2 NeuronCores.

Sharding: pure data-parallel over batch — core b computes batch element b
end-to-end (no collectives).

Causal fast path (the harness case) uses mixed fp8/fp16 precision:
  - Q/K/V/O projections run as fp8e4m3 DoubleRow matmuls (2 k-tiles per
    instruction -> 2x PE throughput vs bf16). Host pre-scales x by 16 and
    weights by 64 (power-of-2) so fp8 values sit in the normal range; the
    descales fold into the existing psum-eviction tensor_scalar ops.
  - scores / exp / attnV run in fp16 (same PE rate as bf16, 8x the mantissa).
  - hi-region: outputs at early rows are large (short causal span -> out ~ v_i)
    while later rows average hundreds of values; absmax error budget is set by
    those early rows. So q columns i<128, k/v positions j<128, and y columns
    i<128 are computed with fp16 weights end-to-end. CPU-emulated end-to-end
    absmax rel err of this exact config: 5.1e-3 (gate 2e-2).
  - softmax normalization rides the attnV matmul: V-tile gets a 65th column
    of 1/8 whose output row is the (scaled) attn row-sum; reciprocal times 8
    is folded so AO comes out x8, in fp8 range; Wo descale absorbs it.

General-mask fallback path is the previous all-bf16 kernel.
"""

import sys
import numpy as np
import ml_dtypes

for _p in ("/opt/trn_rl_repo",):
    if _p not in sys.path:
        sys.path.append(_p)

import concourse.bass as bass
import concourse.mybir as mybir
import concourse.tile as tile
from concourse import bacc
from concourse.bass_utils import run_bass_kernel_spmd

F32 = mybir.dt.float32
F16 = mybir.dt.float16
F8 = mybir.dt.float8e4

B, S, E, H, D = 8, 1500, 1024, 16, 64
P = 128
HI = 128          # hi-precision region width (rows/cols)
NEG = -1e9


def _chunks(total, step):
    return [(c0, min(step, total - c0)) for c0 in range(0, total, step)]


def _wslices(dram_ap, col0, cols):
    """[E, E] weight -> [P, E//P, cols] AP for a column slice (k on partition)."""
    return dram_ap.rearrange("(kt p) f -> p kt f", p=P)[:, :, col0:col0 + cols]


def build_causal():
    KT_N = E // P            # k-tiles over the embedding dim (8)
    FT_N = E // P            # f-tiles (8)
    TP_N = KT_N // 2         # DoubleRow k-tile pairs (4)
    R_CH = _chunks(S, 512)   # i/r chunks
    JB = _chunks(S, P)       # j blocks
    OCH = [(HI, 512 - HI)] + _chunks(S, 512)[1:]  # fp8 y chunks (cols >= HI)
    NRM = F16  # attn row-sums peak ~5e3, well inside fp16; 2x cheaper matmul
    DR = mybir.MatmulPerfMode.DoubleRow
    nc = bacc.Bacc("TRN2", target_bir_lowering=False, debug=False,
                   num_devices=8)

    x8T = nc.dram_tensor("x8T", [E, S], F8, kind="ExternalInput")
    # x8 re-arranged [p, tp, jb, 2, 128] (pairs contiguous) for the V-proj
    # stationary operand: Ldweights requires contiguous DoubleRow pairs.
    NJB = (S + P - 1) // P
    x8v = nc.dram_tensor("x8v", [P, (KT_N // 2) * NJB * 2 * P], F8,
                         kind="ExternalInput")
    x16T = nc.dram_tensor("x16T", [E, HI], F16, kind="ExternalInput")
    wq8T = nc.dram_tensor("wq8T", [E, E], F8, kind="ExternalInput")
    wk8T = nc.dram_tensor("wk8T", [E, E], F8, kind="ExternalInput")
    wv8T = nc.dram_tensor("wv8T", [E, E], F8, kind="ExternalInput")
    wo8T = nc.dram_tensor("wo8T", [E, E], F8, kind="ExternalInput")
    wq16T = nc.dram_tensor("wq16T", [E, E], F16, kind="ExternalInput")
    wk16T = nc.dram_tensor("wk16T", [E, E], F16, kind="ExternalInput")
    wv16T = nc.dram_tensor("wv16T", [E, E], F16, kind="ExternalInput")
    wo16T = nc.dram_tensor("wo16T", [E, E], F16, kind="ExternalInput")
    bq = nc.dram_tensor("bq", [E], F32, kind="ExternalInput")   # bq / 8
    bo = nc.dram_tensor("bo", [E], F32, kind="ExternalInput")   # bo + Wo@bv
    yT = nc.dram_tensor("yT", [E, S], F16, kind="ExternalOutput")

    # psum descales (power-of-2, exact):
    #   fp8 q psum = (16 x)(64 w) = 1024 q ; want q/8 stored -> 2^-13
    #   fp8 k/v psum = 1024 k -> 2^-10
    #   hi  q psum = q -> 2^-3 ; hi k/v -> 1
    #   fp8 y psum = (8 ao)(64 wo) = 512 y -> 2^-9 ; hi y psum = 8 y -> 2^-3
    SQ8, SK8, SQH = 2.0 ** -13, 2.0 ** -10, 0.125
    SY8, SYH = 2.0 ** -9, 0.125

    nc._allow_low_precision_reason = "mixed fp8/fp16 matmul operand chain"
    with tile.TileContext(nc) as tc:
        with (
            tc.tile_pool(name="persist", bufs=1) as pers,
            tc.tile_pool(name="wqk8p", bufs=2) as wqk8p,
            tc.tile_pool(name="wqk16p", bufs=2) as wqk16p,
            tc.tile_pool(name="wvp", bufs=1) as wvp,
            tc.tile_pool(name="wop", bufs=1) as wop,
            tc.tile_pool(name="attn", bufs=3) as apool,
            tc.tile_pool(name="small", bufs=3) as spool,
            tc.tile_pool(name="evp", bufs=3) as evp,
            tc.tile_pool(name="psP", bufs=1, space="PSUM") as psP,
            tc.tile_pool(name="psS", bufs=1, space="PSUM") as psS,
            tc.tile_pool(name="psO", bufs=1, space="PSUM") as psO,
        ):
            ones64 = pers.tile([1, D], NRM, name="ones64")
            nc.vector.memset(ones64[:], 1.0)
            bq_sb = pers.tile([P, FT_N], F32, name="bq_sb")
            nc.sync.dma_start(out=bq_sb[:], in_=bq.ap().rearrange("(t p) -> p t", p=P))
            bo_sb = pers.tile([P, FT_N], F32, name="bo_sb")
            nc.sync.dma_start(out=bo_sb[:], in_=bo.ap().rearrange("(t p) -> p t", p=P))

            # upper-triangular (incl diag) 0/1 mask for diagonal attn blocks
            tri32 = pers.tile([P, P], F32, name="tri32")
            nc.gpsimd.memset(tri32[:], 1.0)
            nc.gpsimd.affine_select(
                out=tri32[:], in_=tri32[:],
                pattern=[[1, P]], compare_op=mybir.AluOpType.is_ge,
                fill=0.0, base=0, channel_multiplier=-1,
            )
            tri = pers.tile([P, P], F16, name="tri")
            nc.vector.tensor_copy(out=tri[:], in_=tri32[:])

            XT8 = pers.tile([P, KT_N, S], F8, name="xt8")
            XV8 = pers.tile([P, TP_N, len(JB), 2, P], F8, name="xv8")
            XT16 = pers.tile([P, KT_N, HI], F16, name="xt16")
            QT = [pers.tile([P, S], F16, name=f"qt{ft}") for ft in range(FT_N)]
            KTs = [pers.tile([P, S], F16, name=f"kt{ft}") for ft in range(FT_N)]
            VA = [pers.tile([P, H * (D + 1)], F16, name=f"va{rt}")
                  for rt in range(len(JB))]
            AOT8 = pers.tile([P, FT_N, S], F8, name="aot8")
            AOT16 = pers.tile([P, FT_N, HI], F16, name="aot16")

            # ---- V projection weights + x loads (interleaved) ----
            hpc = 512 // D  # heads per 512-wide f chunk
            fchunks = _chunks(E, 512)
            wv8_t = [wvp.tile([P, KT_N, 512], F8, name=f"wv8{fc}",
                              tag=f"wv8{fc}") for fc in range(len(fchunks))]
            xv8_src = x8v.ap().rearrange("p (a b c d) -> p a b c d",
                                         a=TP_N, b=len(JB), c=2)
            for jb in (1, 2):
                nc.sync.dma_start(out=XV8[:, :, jb, :, :],
                                  in_=xv8_src[:, :, jb, :, :])
            for kt in range(KT_N):
                for fc, (f0, fw) in enumerate(fchunks):
                    nc.sync.dma_start(
                        out=wv8_t[fc][:, kt, :fw],
                        in_=_wslices(wv8T.ap(), f0, fw)[:, kt, :])
                nc.sync.dma_start(out=XT8[:, kt, :], in_=x8T[kt * P:(kt + 1) * P, :])
                if kt == 1:
                    nc.sync.dma_start(out=XV8[:, :, 3, :, :],
                                      in_=xv8_src[:, :, 3, :, :])
                    nc.sync.dma_start(
                        out=XT16[:],
                        in_=x16T.ap().rearrange("(kt p) s -> p kt s", p=P))
            for jb in range(4, len(JB)):
                nc.sync.dma_start(out=XV8[:, :, jb, :, :],
                                  in_=xv8_src[:, :, jb, :, :])
            wv16_t = [wvp.tile([P, KT_N, 512], F16, name=f"wv16{fc}",
                               tag=f"wv16{fc}") for fc in range(len(fchunks))]
            for fc, (f0, fw) in enumerate(fchunks):
                nc.sync.dma_start(out=wv16_t[fc][:],
                                  in_=_wslices(wv16T.ap(), f0, fw))

            def emit_v8(rts):
                """fp8 V projection for j-blocks >= 1."""
                for rt in rts:
                    if rt >= len(JB) or rt == 0:
                        continue
                    r0, rsz = JB[rt]
                    for fc, (f0, fw) in enumerate(fchunks):
                        ps = psP.tile([P, 512], F32, name="pv", tag="pp", bufs=2)
                        for tp in range(TP_N):
                            nc.tensor.matmul(
                                ps[:, :fw],
                                XV8[:, tp, rt, :, :],
                                wv8_t[fc][:, 2 * tp:2 * tp + 2, :fw],
                                start=(tp == 0), stop=(tp == TP_N - 1),
                                perf_mode=DR,
                            )
                        dst = VA[rt][:].rearrange("p (h c) -> p h c", c=D + 1)
                        nc.vector.tensor_scalar(
                            out=dst[:rsz, fc * hpc:fc * hpc + fw // D, 0:D],
                            in0=ps[:rsz, :fw].rearrange("p (h d) -> p h d", d=D),
                            scalar1=SK8, scalar2=None, op0=mybir.AluOpType.mult,
                        )
                    va3 = VA[rt][:].rearrange("p (h c) -> p h c", c=D + 1)
                    nc.gpsimd.memset(va3[:rsz, :, D:D + 1], 0.125)

            def emit_v_hi():
                """fp16 V projection for j-block 0."""
                rsz = P
                for fc, (f0, fw) in enumerate(fchunks):
                    ps = psP.tile([P, 512], F32, name="pv", tag="pp", bufs=2)
                    for kt in range(KT_N):
                        nc.tensor.matmul(
                            ps[:rsz, :fw],
                            XT16[:, kt, :],
                            wv16_t[fc][:, kt, :fw],
                            start=(kt == 0), stop=(kt == KT_N - 1),
                        )
                    dst = VA[0][:].rearrange("p (h c) -> p h c", c=D + 1)
                    nc.vector.tensor_copy(
                        out=dst[:rsz, fc * hpc:fc * hpc + fw // D, 0:D],
                        in_=ps[:rsz, :fw].rearrange("p (h d) -> p h d", d=D),
                    )
                va3 = VA[0][:].rearrange("p (h c) -> p h c", c=D + 1)
                nc.gpsimd.memset(va3[:rsz, :, D:D + 1], 0.125)

            def proj_qk_gen(ft):
                for which, w8d, w16d, dst in (("q", wq8T, wq16T, QT),
                                              ("k", wk8T, wk16T, KTs)):
                    wt8 = wqk8p.tile([P, KT_N, P], F8, name="wqk8", tag="wqk8")
                    nc.sync.dma_start(out=wt8[:], in_=_wslices(w8d.ap(), ft * P, P))
                    wt16 = wqk16p.tile([P, KT_N, P], F16, name="wqk16",
                                       tag="wqk16")
                    nc.sync.dma_start(out=wt16[:], in_=_wslices(w16d.ap(), ft * P, P))
                    for rc, (c0, cw) in enumerate(R_CH):
                        ps = psP.tile([P, 512], F32, name="pp", tag="pp", bufs=2)
                        for tp in range(TP_N):
                            nc.tensor.matmul(
                                ps[:, :cw],
                                wt8[:, 2 * tp:2 * tp + 2, :],
                                XT8[:, 2 * tp:2 * tp + 2, c0:c0 + cw],
                                start=(tp == 0), stop=(tp == TP_N - 1),
                                perf_mode=DR,
                            )
                        lo = HI if rc == 0 else 0  # cols < HI come from hi path
                        if which == "q":
                            nc.vector.tensor_scalar(
                                out=dst[ft][:, c0 + lo:c0 + cw],
                                in0=ps[:, lo:cw],
                                scalar1=SQ8, scalar2=bq_sb[:, ft:ft + 1],
                                op0=mybir.AluOpType.mult,
                                op1=mybir.AluOpType.add,
                            )
                        else:
                            nc.vector.tensor_scalar(
                                out=dst[ft][:, c0 + lo:c0 + cw],
                                in0=ps[:, lo:cw],
                                scalar1=SK8, scalar2=None,
                                op0=mybir.AluOpType.mult,
                            )
                        yield
                    # hi group: cols [0, HI) with fp16 operands
                    ps = psP.tile([P, 512], F32, name="pp", tag="pp", bufs=2)
                    for kt in range(KT_N):
                        nc.tensor.matmul(
                            ps[:, :HI],
                            wt16[:, kt, :],
                            XT16[:, kt, :],
                            start=(kt == 0), stop=(kt == KT_N - 1),
                        )
                    if which == "q":
                        nc.vector.tensor_scalar(
                            out=dst[ft][:, 0:HI], in0=ps[:, :HI],
                            scalar1=SQH, scalar2=bq_sb[:, ft:ft + 1],
                            op0=mybir.AluOpType.mult, op1=mybir.AluOpType.add,
                        )
                    else:
                        nc.vector.tensor_copy(out=dst[ft][:, 0:HI],
                                              in_=ps[:, :HI])
                    yield

            def proj_qk(ft):
                for _ in proj_qk_gen(ft):
                    pass

            def attn_ft(ic, ft, filler=None):
                c0, cw = R_CH[ic]
                nblk = min(len(JB), (c0 + cw + P - 1) // P)
                pso = [psO.tile([D + 1, 512], F32, name=f"po{half}",
                                tag="po", bufs=2)
                       for half in range(2)]
                # diagonal-containing blocks first so the chunk-end attnV
                # gates on a short (non-masked) exp chain
                cut = max(0, nblk - (cw + P - 1) // P)
                order = list(range(cut, nblk)) + list(range(cut))
                for n_i, jb in enumerate(order):
                    j0, jsz = JB[jb]
                    vo = max(0, j0 - c0)
                    # both halves' scores land in one 2-bank psum pair so a
                    # single ACTIVATE exps them together (halves ACT op count)
                    psp = psS.tile([P, 2, 512], F32, name="psp",
                                   tag="ps", bufs=2)
                    for half in range(2):
                        d0 = D * half
                        nc.tensor.matmul(
                            psp[:jsz, half, vo:cw],
                            KTs[ft][d0:d0 + D, j0:j0 + jsz],
                            QT[ft][d0:d0 + D, c0 + vo:c0 + cw],
                            start=True, stop=True,
                            tile_position=(d0, 0),
                        )
                    atp = apool.tile([P, 2, 512], F16, name="atp")
                    nc.scalar.activation(
                        out=atp[:jsz, :, vo:cw], in_=psp[:jsz, :, vo:cw],
                        func=mybir.ActivationFunctionType.Exp,
                    )
                    if j0 >= c0:
                        # zero attn where j > i on the diagonal square
                        for half in range(2):
                            nc.vector.tensor_tensor(
                                out=atp[:jsz, half, vo:vo + jsz],
                                in0=atp[:jsz, half, vo:vo + jsz],
                                in1=tri[:jsz, :jsz],
                                op=mybir.AluOpType.mult,
                            )
                    va3 = VA[jb][:].rearrange("p (h c) -> p h c", c=D + 1)
                    for half in range(2):
                        nc.tensor.matmul(
                            pso[half][:, vo:cw],
                            va3[:jsz, 2 * ft + half, :],
                            atp[:jsz, half, vo:cw],
                            start=(n_i == 0), stop=(n_i == nblk - 1),
                        )
                    if filler is not None and n_i % 4 == 3:
                        filler()
                ssums = []
                for half in range(2):
                    ssum = spool.tile([1, 512], NRM, name=f"ssum{half}",
                                      tag="ssum")
                    nc.vector.tensor_copy(
                        out=ssum[:, :cw], in_=pso[half][D:D + 1, :cw])
                    ssums.append(ssum)
                for half in range(2):
                    d0 = D * half
                    psb = psP.tile([D, 512], F32, name="psb", tag="pp", bufs=2)
                    nc.tensor.matmul(
                        psb[:, :cw], ones64[:, :], ssums[half][:, :cw],
                        start=True, stop=True,
                    )
                    rb = spool.tile([D, 512], F32, name="rb")
                    nc.vector.reciprocal_approx_fast(
                        out=rb[:, :cw], in_=psb[:, :cw])
                    # AO x8 (ones col = 1/8 makes rb = 8/sum)
                    nc.vector.tensor_tensor(
                        out=AOT8[d0:d0 + D, ft, c0:c0 + cw],
                        in0=pso[half][0:D, :cw], in1=rb[:, :cw],
                        op=mybir.AluOpType.mult,
                    )
                    if ic == 0:
                        nc.vector.tensor_tensor(
                            out=AOT16[d0:d0 + D, ft, 0:HI],
                            in0=pso[half][0:D, 0:HI], in1=rb[:, 0:HI],
                            op=mybir.AluOpType.mult,
                        )

            def emit_yt8(ot, oc, wo_t):
                c0, cw = OCH[oc]
                psy = psP.tile([P, 512], F32, name="py", tag="pp", bufs=2)
                for tp in range(TP_N):
                    nc.tensor.matmul(
                        psy[:, :cw],
                        wo_t[:, 2 * tp:2 * tp + 2, :],
                        AOT8[:, 2 * tp:2 * tp + 2, c0:c0 + cw],
                        start=(tp == 0), stop=(tp == TP_N - 1),
                        perf_mode=DR,
                    )
                yt = evp.tile([P, 512], F16, name="yt", tag="yt")
                nc.vector.tensor_scalar(
                    out=yt[:, :cw], in0=psy[:, :cw],
                    scalar1=SY8, scalar2=bo_sb[:, ot:ot + 1],
                    op0=mybir.AluOpType.mult, op1=mybir.AluOpType.add,
                )
                nc.sync.dma_start(
                    out=yT[ot * P:(ot + 1) * P, c0:c0 + cw], in_=yt[:, :cw])

            def emit_yt_hi(ot, wo16_t):
                psy = psP.tile([P, 512], F32, name="py", tag="pp", bufs=2)
                for ft in range(FT_N):
                    nc.tensor.matmul(
                        psy[:, :HI],
                        wo16_t[:, ft, :],
                        AOT16[:, ft, :],
                        start=(ft == 0), stop=(ft == FT_N - 1),
                    )
                yt = evp.tile([P, 512], F16, name="yt", tag="yt")
                nc.vector.tensor_scalar(
                    out=yt[:, :HI], in0=psy[:, :HI],
                    scalar1=SYH, scalar2=bo_sb[:, ot:ot + 1],
                    op0=mybir.AluOpType.mult, op1=mybir.AluOpType.add,
                )
                nc.sync.dma_start(out=yT[ot * P:(ot + 1) * P, 0:HI],
                                  in_=yt[:, :HI])

            # ---- schedule ----
            nb0 = min(len(JB), (R_CH[0][0] + R_CH[0][1] + P - 1) // P)
            emit_v8(range(1, nb0))
            proj_qk(0)
            emit_v_hi()
            wo8_tiles = []
            wo16_tiles = []

            def load_wo(lst, dram, dt, pfx):
                for ot in range(FT_N):
                    wt = wop.tile([P, KT_N, P], dt, name=f"{pfx}{ot}",
                                  tag=f"{pfx}{ot}")
                    nc.sync.dma_start(out=wt[:], in_=_wslices(dram.ap(), ot * P, P))
                    lst.append(wt)

            nbp = nb0
            for ft in range(FT_N):
                if ft == 2:
                    load_wo(wo8_tiles, wo8T, F8, "wo8")
                if ft == 3:
                    load_wo(wo16_tiles, wo16T, F16, "wo16")
                gen = proj_qk_gen(ft + 1) if ft + 1 < FT_N else None

                def pump():
                    if gen is not None:
                        next(gen, None)

                for ic in range(len(R_CH)):
                    attn_ft(ic, ft, filler=pump)
                    if ft == 0 and ic + 1 < len(R_CH):
                        c0n, cwn = R_CH[ic + 1]
                        nbn = min(len(JB), (c0n + cwn + P - 1) // P)
                        emit_v8(range(nbp, nbn))
                        nbp = nbn
                    if ft == FT_N - 1:
                        # last ft has no next-ft projection filler: use the
                        # now-ready yT chunk as PE filler instead
                        for ot in range(FT_N):
                            if ic == 0:
                                emit_yt_hi(ot, wo16_tiles[ot])
                            emit_yt8(ot, ic, wo8_tiles[ot])
                if gen is not None:
                    for _ in gen:
                        pass

    nc.compile()
    return nc


def build_general(mm_dt=mybir.dt.bfloat16):
    """All-bf16 fallback for a non-causal additive mask (not the harness
    case). Same as the previous baseline kernel's general path."""
    KT_N = E // P
    FT_N = E // P
    R_CH = _chunks(S, 512)
    JB = _chunks(S, P)
    nc = bacc.Bacc("TRN2", target_bir_lowering=False, debug=False,
                   num_devices=8)
    MD = mm_dt
    NRM = mybir.dt.float32r

    xT = nc.dram_tensor("xT", [E, S], MD, kind="ExternalInput")
    wqT = nc.dram_tensor("wqT", [E, E], MD, kind="ExternalInput")
    wkT = nc.dram_tensor("wkT", [E, E], MD, kind="ExternalInput")
    wvT = nc.dram_tensor("wvT", [E, E], MD, kind="ExternalInput")
    woT = nc.dram_tensor("woT", [E, E], MD, kind="ExternalInput")
    bq = nc.dram_tensor("bq", [E], F32, kind="ExternalInput")
    bo = nc.dram_tensor("bo", [E], F32, kind="ExternalInput")
    maskT = nc.dram_tensor("maskT", [S, S], F32, kind="ExternalInput")
    yT = nc.dram_tensor("yT", [E, S], F32, kind="ExternalOutput")

    nc._allow_low_precision_reason = "low-precision matmul operand chain"
    with tile.TileContext(nc) as tc:
        with (
            tc.tile_pool(name="persist", bufs=1) as pers,
            tc.tile_pool(name="wqkp", bufs=2) as wqkp,
            tc.tile_pool(name="wvp", bufs=1) as wvp,
            tc.tile_pool(name="wop", bufs=1) as wop,
            tc.tile_pool(name="attn", bufs=3) as apool,
            tc.tile_pool(name="small", bufs=3) as spool,
            tc.tile_pool(name="evp", bufs=3) as evp,
            tc.tile_pool(name="psP", bufs=1, space="PSUM") as psP,
            tc.tile_pool(name="psS", bufs=1, space="PSUM") as psS,
            tc.tile_pool(name="psO", bufs=1, space="PSUM") as psO,
        ):
            ones64 = pers.tile([1, D], NRM, name="ones64")
            nc.vector.memset(ones64[:].bitcast(F32), 1.0)
            bq_sb = pers.tile([P, FT_N], F32, name="bq_sb")
            nc.sync.dma_start(out=bq_sb[:], in_=bq.ap().rearrange("(t p) -> p t", p=P))
            bo_sb = pers.tile([P, FT_N], F32, name="bo_sb")
            nc.sync.dma_start(out=bo_sb[:], in_=bo.ap().rearrange("(t p) -> p t", p=P))

            XT = [pers.tile([P, S], MD, name=f"xt{kt}") for kt in range(KT_N)]
            QT = [pers.tile([P, S], MD, name=f"qt{ft}") for ft in range(FT_N)]
            KTs = [pers.tile([P, S], MD, name=f"kt{ft}") for ft in range(FT_N)]
            VA = [pers.tile([P, H * (D + 1)], MD, name=f"va{rt}")
                  for rt in range(len(JB))]
            AOT = [pers.tile([P, S], MD, name=f"aot{ft}") for ft in range(FT_N)]

            hpc = 512 // D
            fchunks = _chunks(E, 512)
            wv_tiles = [wvp.tile([P, KT_N, 512], MD, name=f"wv{fc}", tag=f"wv{fc}")
                        for fc in range(len(fchunks))]
            for kt in range(KT_N):
                for fc, (f0, fw) in enumerate(fchunks):
                    nc.sync.dma_start(
                        out=wv_tiles[fc][:, kt, :fw],
                        in_=_wslices(wvT.ap(), f0, fw)[:, kt, :])
                nc.sync.dma_start(out=XT[kt][:],
                                  in_=xT[kt * P:(kt + 1) * P, :])

            def emit_v(rts):
                for rt in rts:
                    if rt >= len(JB):
                        continue
                    r0, rsz = JB[rt]
                    for fc, (f0, fw) in enumerate(fchunks):
                        wt = wv_tiles[fc]
                        ps = psP.tile([P, 512], F32, name="pv", tag="pp", bufs=2)
                        for kt in range(KT_N):
                            nc.tensor.matmul(
                                ps[:rsz, :fw],
                                XT[kt][:, r0:r0 + rsz],
                                wt[:, kt, :fw],
                                start=(kt == 0), stop=(kt == KT_N - 1),
                            )
                        dst = VA[rt][:].rearrange("p (h c) -> p h c", c=D + 1)
                        nc.vector.tensor_copy(
                            out=dst[:rsz, fc * hpc:fc * hpc + fw // D, 0:D],
                            in_=ps[:rsz, :fw].rearrange("p (h d) -> p h d", d=D),
                        )
                    va3 = VA[rt][:].rearrange("p (h c) -> p h c", c=D + 1)
                    nc.gpsimd.memset(va3[:rsz, :, D:D + 1], 1.0)

            def proj_qk(ft):
                for which, wdram, dst in (("q", wqT, QT), ("k", wkT, KTs)):
                    wt = wqkp.tile([P, KT_N, P], MD, name="wqk", tag="wqk")
                    nc.sync.dma_start(out=wt[:], in_=_wslices(wdram.ap(), ft * P, P))
                    for rc, (c0, cw) in enumerate(R_CH):
                        ps = psP.tile([P, 512], F32, name="pp", tag="pp", bufs=2)
                        for kt in range(KT_N):
                            nc.tensor.matmul(
                                ps[:, :cw],
                                wt[:, kt, :],
                                XT[kt][:, c0:c0 + cw],
                                start=(kt == 0), stop=(kt == KT_N - 1),
                            )
                        if which == "q":
                            nc.vector.tensor_scalar(
                                out=dst[ft][:, c0:c0 + cw], in0=ps[:, :cw],
                                scalar1=bq_sb[:, ft:ft + 1], scalar2=None,
                                op0=mybir.AluOpType.add,
                            )
                        else:
                            nc.vector.tensor_copy(
                                out=dst[ft][:, c0:c0 + cw], in_=ps[:, :cw])

            def attn_ft(ic, ft, mtiles):
                c0, cw = R_CH[ic]
                nblk = len(JB)
                pso = [psO.tile([D + 1, 512], F32, name=f"po{half}",
                                tag="po", bufs=2)
                       for half in range(2)]
                for n_i, jb in enumerate(range(nblk)):
                    j0, jsz = JB[jb]
                    psp = psS.tile([P, 2, 512], F32, name="psp",
                                   tag="ps", bufs=2)
                    for half in range(2):
                        d0 = D * half
                        nc.tensor.matmul(
                            psp[:jsz, half, :cw],
                            KTs[ft][d0:d0 + D, j0:j0 + jsz],
                            QT[ft][d0:d0 + D, c0:c0 + cw],
                            start=True, stop=True,
                            tile_position=(d0, 0),
                        )
                    for half in range(2):
                        nc.vector.tensor_tensor(
                            out=psp[:jsz, half, :cw],
                            in0=psp[:jsz, half, :cw],
                            in1=mtiles[jb][:jsz, :cw],
                            op=mybir.AluOpType.add,
                        )
                    atp = apool.tile([P, 2, 512], MD, name="atp")
                    nc.scalar.activation(
                        out=atp[:jsz, :, :cw], in_=psp[:jsz, :, :cw],
                        func=mybir.ActivationFunctionType.Exp,
                    )
                    va3 = VA[jb][:].rearrange("p (h c) -> p h c", c=D + 1)
                    for half in range(2):
                        nc.tensor.matmul(
                            pso[half][:, :cw],
                            va3[:jsz, 2 * ft + half, :],
                            atp[:jsz, half, :cw],
                            start=(n_i == 0), stop=(n_i == nblk - 1),
                        )
                ssums = []
                for half in range(2):
                    ssum = spool.tile([1, 512], NRM, name=f"ssum{half}",
                                      tag="ssum")
                    nc.vector.tensor_copy(
                        out=ssum[:, :cw], in_=pso[half][D:D + 1, :cw])
                    ssums.append(ssum)
                for half in range(2):
                    d0 = D * half
                    psb = psP.tile([D, 512], F32, name="psb", tag="pp", bufs=2)
                    nc.tensor.matmul(
                        psb[:, :cw], ones64[:, :], ssums[half][:, :cw],
                        start=True, stop=True,
                    )
                    rb = spool.tile([D, 512], F32, name="rb")
                    nc.vector.reciprocal_approx_fast(
                        out=rb[:, :cw], in_=psb[:, :cw])
                    nc.vector.tensor_tensor(
                        out=AOT[ft][d0:d0 + D, c0:c0 + cw],
                        in0=pso[half][0:D, :cw], in1=rb[:, :cw],
                        op=mybir.AluOpType.mult,
                    )

            def emit_yt(ot, rc, wo_t):
                c0, cw = R_CH[rc]
                psy = psP.tile([P, 512], F32, name="py", tag="pp", bufs=2)
                for ft in range(FT_N):
                    nc.tensor.matmul(
                        psy[:, :cw],
                        wo_t[:, ft, :],
                        AOT[ft][:, c0:c0 + cw],
                        start=(ft == 0), stop=(ft == FT_N - 1),
                    )
                yt = evp.tile([P, 512], F16, name="yt", tag="yt")
                nc.vector.tensor_scalar(
                    out=yt[:, :cw], in0=psy[:, :cw],
                    scalar1=bo_sb[:, ot:ot + 1], scalar2=None,
                    op0=mybir.AluOpType.add,
                )
                nc.sync.dma_start(
                    out=yT[ot * P:(ot + 1) * P, c0:c0 + cw], in_=yt[:, :cw])

            emit_v(range(len(JB)))
            for ft in range(FT_N):
                proj_qk(ft)
            with tc.tile_pool(name="maskp", bufs=1) as mpool:
                for ic, (c0, cw) in enumerate(R_CH):
                    mtiles = []
                    for jb, (j0, jsz) in enumerate(JB):
                        mt = mpool.tile([P, 512], F32, name=f"m{jb}")
                        nc.sync.dma_start(
                            out=mt[:jsz, :cw],
                            in_=maskT[j0:j0 + jsz, c0:c0 + cw])
                        mtiles.append(mt)
                    for ft in range(FT_N):
                        attn_ft(ic, ft, mtiles)
            for ot in range(FT_N):
                wt = wop.tile([P, KT_N, P], MD, name=f"wo{ot}", tag="wo",
                              bufs=2)
                nc.sync.dma_start(out=wt[:], in_=_wslices(woT.ap(), ot * P, P))
                for rc in range(len(R_CH)):
                    emit_yt(ot, rc, wt)

    nc.compile()
    return nc


_CACHE: dict = {}


def _get_nc(causal: bool):
    if causal not in _CACHE:
        _CACHE[causal] = build_causal() if causal else build_general()
    return _CACHE[causal]


def _is_causal(mask: np.ndarray) -> bool:
    if mask.shape != (S, S):
        return False
    expect = np.where(np.tril(np.ones((S, S), dtype=bool)), np.float32(0.0),
                      np.float32(NEG))
    return bool(np.array_equal(mask, expect))


F8_NP = ml_dtypes.float8_e4m3
F16_NP = np.float16
BF16_NP = ml_dtypes.bfloat16


def _f8(a, s):
    return np.clip(np.asarray(a, np.float32) * np.float32(s),
                   -240.0, 240.0).astype(F8_NP)


def _x8v(x8):
    """[E, S] fp8 -> [P, tp*jb*2*128] with DoubleRow pairs contiguous."""
    njb = (S + P - 1) // P
    xp = np.zeros((E, njb * P), F8_NP)
    xp[:, :S] = x8
    v = xp.reshape(4, 2, P, njb, P)   # (tp, k2, p, jb, c)
    v = v.transpose(2, 0, 3, 1, 4)    # (p, tp, jb, k2, c)
    return np.ascontiguousarray(v.reshape(P, -1))


def prep_inputs(x, mask, Wq, bq, Wk, Wv, bv, Wo, bo):
    """Host-side preprocessing shared by kernel() and the bench harness."""
    scale = np.float32(1.0 / np.sqrt(D))
    causal = _is_causal(np.asarray(mask))
    if causal:
        xT = np.ascontiguousarray(
            np.transpose(np.asarray(x, np.float32), (0, 2, 1)))
        common = {
            "wq8T": np.ascontiguousarray(_f8(Wq.T, 64.0)),
            "wk8T": np.ascontiguousarray(_f8(Wk.T, 64.0)),
            "wv8T": np.ascontiguousarray(_f8(Wv.T, 64.0)),
            "wo8T": np.ascontiguousarray(_f8(Wo.T, 64.0)),
            "wq16T": np.ascontiguousarray(np.asarray(Wq.T, np.float32)).astype(F16_NP),
            "wk16T": np.ascontiguousarray(np.asarray(Wk.T, np.float32)).astype(F16_NP),
            "wv16T": np.ascontiguousarray(np.asarray(Wv.T, np.float32)).astype(F16_NP),
            "wo16T": np.ascontiguousarray(np.asarray(Wo.T, np.float32)).astype(F16_NP),
            "bq": (np.asarray(bq, np.float32) * scale),
            "bo": (np.asarray(bo, np.float32)
                   + np.asarray(Wo, np.float32) @ np.asarray(bv, np.float32)),
        }
        in_maps = []
        for b in range(B):
            x8 = _f8(xT[b], 16.0)
            in_maps.append(dict(
                common, x8T=x8, x8v=_x8v(x8),
                x16T=np.ascontiguousarray(xT[b][:, :HI]).astype(F16_NP)))
        return causal, in_maps
    xTb = np.ascontiguousarray(
        np.transpose(np.asarray(x, np.float32), (0, 2, 1))).astype(BF16_NP)
    common = {
        "wqT": np.ascontiguousarray((np.asarray(Wq, np.float32) * scale).T).astype(BF16_NP),
        "wkT": np.ascontiguousarray(np.asarray(Wk, np.float32).T).astype(BF16_NP),
        "wvT": np.ascontiguousarray(np.asarray(Wv, np.float32).T).astype(BF16_NP),
        "woT": np.ascontiguousarray(np.asarray(Wo, np.float32).T).astype(BF16_NP),
        "bq": (np.asarray(bq, np.float32) * scale),
        "bo": (np.asarray(bo, np.float32)
               + np.asarray(Wo, np.float32) @ np.asarray(bv, np.float32)),
        "maskT": np.ascontiguousarray(np.asarray(mask, np.float32).T),
    }
    in_maps = [dict(common, xT=xTb[b]) for b in range(B)]
    return causal, in_maps


_RUNNER: dict = {}


def _get_runner(causal: bool):
    """Compile once per mask-variant; cache the jitted SPMD executable."""
    if causal in _RUNNER:
        return _RUNNER[causal]
    import jax
    from jax.sharding import Mesh, PartitionSpec, NamedSharding
    import warnings
    with warnings.catch_warnings():
        warnings.simplefilter("ignore")
        from jax.experimental.shard_map import shard_map
    from concourse import bass2jax
    from concourse.bass2jax import _bass_exec_p, install_neuronx_cc_hook

    nc = _get_nc(causal)
    install_neuronx_cc_hook()
    partition_name = (nc.partition_id_tensor.name
                      if nc.partition_id_tensor else None)
    in_names, out_names, out_avals = [], [], []
    for alloc in nc.m.functions[0].allocations:
        if not isinstance(alloc, mybir.MemoryLocationSet):
            continue
        name = alloc.memorylocations[0].name
        if alloc.kind == "ExternalInput":
            if name != partition_name:
                in_names.append(name)
        elif alloc.kind == "ExternalOutput":
            out_names.append(name)
            out_avals.append(jax.core.ShapedArray(
                tuple(alloc.tensor_shape), mybir.dt.np(alloc.dtype)))
    n_params = len(in_names)
    n_outs = len(out_names)

    def _body(*args):
        operands = list(args)
        names = list(in_names) + list(out_names)
        if partition_name is not None:
            operands.append(bass2jax.partition_id_tensor())
            names.append(partition_name)
        outs = _bass_exec_p.bind(
            *operands,
            out_avals=tuple(out_avals),
            in_names=tuple(names),
            out_names=tuple(out_names),
            lowering_input_output_aliases=(),
            sim_require_finite=True,
            sim_require_nnan=True,
            nc=nc,
        )
        return tuple(outs)

    devices = jax.devices()[:B]
    mesh = Mesh(np.asarray(devices), ("core",))
    in_specs = (PartitionSpec("core"),) * (n_params + n_outs)
    out_specs = (PartitionSpec("core"),) * n_outs
    fn = jax.jit(
        shard_map(_body, mesh=mesh, in_specs=in_specs, out_specs=out_specs,
                  check_rep=False),
        donate_argnums=tuple(range(n_params, n_params + n_outs)),
        keep_unused=True,
    )
    runner = (fn, in_names, out_names, out_avals)
    _RUNNER[causal] = runner
    return runner


def kernel(x, mask, Wq, bq, Wk, Wv, bv, Wo, bo):
    causal, in_maps = prep_inputs(x, mask, Wq, bq, Wk, Wv, bv, Wo, bo)
    fn, in_names, out_names, out_avals = _get_runner(causal)
    cat = [np.concatenate([np.asarray(m[n]) for m in in_maps], axis=0)
           for n in in_names]
    zs = [np.zeros((B * a.shape[0], *a.shape[1:]), a.dtype) for a in out_avals]
    outs = fn(*cat, *zs)
    yT = np.asarray(outs[out_names.index("yT")]).reshape(B, E, S)
    out = np.ascontiguousarray(yT.transpose(0, 2, 1).astype(np.float32))
    return out
